# revision 21
# baseline (speedup 1.0000x reference)
"""AnchorStripeAttention Trainium2 kernel (8 NeuronCores, data-parallel).

v2: engine-rebalanced rewrite of the baseline.
  - Shard over batch (2) x window-row-blocks (4): core c -> batch c//4,
    pixel rows [ (c%4)*64, (c%4+1)*64 ) -> 16384 contiguous tokens of qkv,
    32 contiguous rows of the 128-row anchor image. 256 windows per core.
  - CPB-MLP bias tables + logit scales precomputed on host (params only).
  - Anchors: loaded once [128, 32, 192] (partition = wr*16 + i*4 + j),
    l2-normalized densely ONCE per core (baseline redid this per chunk at
    16/128 occupancy).
  - Per 2-window chunk: qk squares, q/k scale-mul and e1-mul on GpSimd
    (idle in baseline); reductions/recips/psum-evac on Vector; exps+sqrt
    on Scalar; matmuls ordered so consecutive PE instructions in
    different array tiles always hit different PSUM banks.
  PSUM map (8 banks):
    pa[128,4,512] banks0-3: attn1T (b0,b1 cols0:96, bank=w),
      attn2 (b0-3 cols128:256, bank=h%4 as baseline),
      x1e (b2,b3 cols256:454, bank=2+w)
    oall[128,2,512] banks4-5 (bank=4+w)
    tpA[128,512] bank6 (q/k transposes), tpB[128,64] bank7 (anc transposes)
  Softmax max-subtraction skipped (logits bounded ~26); masks all-zero.
"""

import math
import numpy as np

B = 2
H = 256
WID = 256
NH = 6
DH = 32
CO = NH * DH
C = 3 * CO
WS = 8
AWS = 4
T = 121
NCORES = 8
NWR = 8         # window-rows per core
NWC = 32        # window-cols
TOK = 16384     # tokens per core shard

_NC_CACHE = {}


def _build_nc():
    import concourse.bass as bass
    from concourse import bacc
    import concourse.mybir as mybir
    from concourse.tile import TileContext
    from concourse.masks import make_identity

    f32 = mybir.dt.float32
    AX = mybir.AxisListType
    OP = mybir.AluOpType
    AF = mybir.ActivationFunctionType

    nc = bacc.Bacc("TRN2")
    qkv_d = nc.declare_dram_parameter("qkv", [NWR, WS, NWC, WS, C], f32, isOutput=False)
    anc_d = nc.declare_dram_parameter("anc", [NWR, AWS, NWC, AWS, CO], f32, isOutput=False)
    eb1_d = nc.declare_dram_parameter("eb1", [128, 96], f32, isOutput=False)
    eb2_d = nc.declare_dram_parameter("eb2t", [48, 384], f32, isOutput=False)
    scl_d = nc.declare_dram_parameter("scl", [128, 12], f32, isOutput=False)
    out_d = nc.declare_dram_parameter("out", [NWR, WS, NWC, WS, CO], f32, isOutput=True)

    with TileContext(nc) as tc:
        with (
            tc.tile_pool(name="const", bufs=1) as cpool,
            tc.tile_pool(name="io", bufs=3) as iopool,
            tc.tile_pool(name="work", bufs=2) as wpool,
            tc.tile_pool(name="small", bufs=3) as spool,
            tc.tile_pool(name="ps_pa", bufs=1, space="PSUM") as ps_pa,
            tc.tile_pool(name="ps_o", bufs=1, space="PSUM") as ps_o,
            tc.tile_pool(name="ps_tp", bufs=1, space="PSUM") as ps_tp,
        ):
            ident = cpool.tile([128, 128], f32)
            make_identity(nc, ident)
            eb1_t = cpool.tile([128, 96], f32)
            nc.sync.dma_start(out=eb1_t, in_=eb1_d[:, :])
            eb2_t = cpool.tile([48, 384], f32)
            nc.sync.dma_start(out=eb2_t, in_=eb2_d[:, :])
            scl_t = cpool.tile([128, 12], f32)
            nc.sync.dma_start(out=scl_t, in_=scl_d[:, :])

            # ---- anchors: load all, l2-normalize once, densely ----
            anc_all = cpool.tile([128, NWC, 192], f32)
            for wr in range(NWR):
                for i in range(AWS):
                    nc.sync.dma_start(
                        out=anc_all[16 * wr + 4 * i:16 * wr + 4 * i + 4, :, :],
                        in_=anc_d[wr, i, :, :, :].rearrange("w j c -> j w c"))
            asq = cpool.tile([128, NWC, 192], f32)
            nc.vector.tensor_mul(out=asq, in0=anc_all, in1=anc_all)
            ssA = cpool.tile([128, NWC * 6], f32)
            nc.vector.tensor_reduce(
                out=ssA,
                in_=asq.rearrange("p w (h c) -> p (w h) c", c=32),
                axis=AX.X, op=OP.add)
            rssA = cpool.tile([128, NWC * 6], f32)
            nc.vector.reciprocal(out=rssA, in_=ssA)
            rsA = cpool.tile([128, NWC * 6], f32)
            nc.scalar.sqrt(out=rsA, in_=rssA)
            anc_n = cpool.tile([128, NWC, 192], f32)
            nc.vector.tensor_mul(
                out=anc_n.rearrange("p w (h c) -> p w h c", c=32),
                in0=anc_all.rearrange("p w (h c) -> p w h c", c=32),
                in1=rsA.rearrange("p (w h) -> p w h", h=6)[:, :, :, None
                    ].broadcast_to((128, NWC, 6, 32)))

            # ---- persistent PSUM tiles (8 banks total), zeroed once so
            # the padded single-instruction views read deterministic data
            pa = ps_pa.tile([128, 4, 512], f32, tag="pa")
            nc.vector.memset(pa, 0.0)
            oall = ps_o.tile([128, 2, 512], f32, tag="oall")
            nc.vector.memset(oall, 0.0)
            tpA = ps_tp.tile([128, 512], f32, tag="tpA")
            nc.vector.memset(tpA[64:128, 256:512], 0.0)
            tpB = ps_tp.tile([128, 128], f32, tag="tpB")

            chunks = [(qq, 2 * cc, ss)
                      for qq in range(NWR // 2)
                      for cc in range(NWC // 2)
                      for ss in range(2)]

            st = {}

            def pair_prep(i):
                # anchor staging + transposes for a row-pair; shared by
                # the pair's chunks via the aT tag (bufs must cover reuse)
                q, c0, sub = chunks[i]
                if sub != 0:
                    return st[("aT", q, c0)]
                anc_st = wpool.tile([32, 2, 192], f32, tag="anc_st")
                nc.sync.dma_start(
                    out=anc_st, in_=anc_n[32 * q:32 * q + 32, c0:c0 + 2, :])
                for w in range(2):
                    nc.tensor.transpose(
                        out=tpB[0:128, 32 * w:32 * w + 32],
                        in_=anc_st[0:32, w, 0:128],
                        identity=ident[0:32, 0:32])
                    nc.tensor.transpose(
                        out=tpB[0:64, 64 + 32 * w:64 + 32 * w + 32],
                        in_=anc_st[0:32, w, 128:192],
                        identity=ident[0:32, 0:32])
                aT = wpool.tile([128, 128], f32, tag="aT")
                nc.vector.tensor_copy(out=aT[:, 0:64], in_=tpB[:, 0:64])
                nc.vector.tensor_copy(
                    out=aT[0:64, 64:128], in_=tpB[0:64, 64:128])
                st[("aT", q, c0)] = aT
                return aT

            def front_pre(i):
                q, c0, sub = chunks[i]
                wr = 2 * q + sub
                qk6 = iopool.tile([128, 576], f32, tag="qk6")
                for w in range(2):
                    nc.sync.dma_start(
                        out=qk6[64 * w:64 * w + 64, :],
                        in_=qkv_d[wr, :, c0 + w, :, :])
                v2 = iopool.tile([128, 6, 33], f32, tag="v2")
                nc.gpsimd.memset(v2[:, :, 32:33], 1.0)
                nc.gpsimd.tensor_copy(
                    out=v2[:, :, 0:32],
                    in_=qk6[:, 384:576].rearrange("p (h c) -> p h c", c=32))
                sq = wpool.tile([128, 384], f32, tag="sq")
                nc.gpsimd.tensor_mul(
                    out=sq, in0=qk6[:, 0:384], in1=qk6[:, 0:384])
                st[("qk6", i)] = qk6
                st[("v2", i)] = v2
                st[("sq", i)] = sq

            def front_mid(i):
                qk6 = st[("qk6", i)]
                sq = st.pop(("sq", i))
                ss = spool.tile([128, 12], f32, tag="ss")
                nc.vector.tensor_reduce(
                    out=ss, in_=sq.rearrange("p (h c) -> p h c", c=32),
                    axis=AX.X, op=OP.add)
                rss = spool.tile([128, 12], f32, tag="rss")
                nc.vector.reciprocal(out=rss, in_=ss)
                rs = spool.tile([128, 12], f32, tag="rs")
                nc.scalar.sqrt(out=rs, in_=rss)
                r = spool.tile([128, 12], f32, tag="r")
                nc.vector.tensor_mul(out=r, in0=rs, in1=scl_t)
                qks = wpool.tile([128, 384], f32, tag="qks")
                nc.gpsimd.tensor_mul(
                    out=qks.rearrange("p (h c) -> p h c", c=32),
                    in0=qk6[:, 0:384].rearrange("p (h c) -> p h c", c=32),
                    in1=r[:, :, None].broadcast_to((128, 12, 32)))
                st[("qks", i)] = qks

            def front_pe(i):
                aT = pair_prep(i)
                qks = st.pop(("qks", i))
                nc.tensor.transpose(
                    out=tpA[:, 0:128], in_=qks[:, 0:128], identity=ident)
                nc.tensor.transpose(
                    out=tpA[:, 128:256], in_=qks[:, 192:320], identity=ident)
                nc.tensor.transpose(
                    out=tpA[0:64, 256:384], in_=qks[:, 128:192],
                    identity=ident)
                nc.tensor.transpose(
                    out=tpA[0:64, 384:512], in_=qks[:, 320:384],
                    identity=ident)
                qkT = wpool.tile([128, 512], f32, tag="qkT")
                nc.vector.tensor_copy(out=qkT, in_=tpA)
                st[("qkT", i)] = qkT
                st[("aTc", i)] = aT

            def ax_of(i, w, h):
                q, c0, sub = chunks[i]
                aT = st[("aTc", i)]
                if h < 4:
                    return aT[32 * h:32 * h + 32,
                              32 * w + 16 * sub:32 * w + 16 * sub + 16]
                return aT[32 * (h - 4):32 * (h - 4) + 32,
                          64 + 32 * w + 16 * sub:64 + 32 * w + 16 * sub + 16]

            def qx_of(i, w, h):
                qkT = st[("qkT", i)]
                if h < 4:
                    return qkT[32 * h:32 * h + 32, 64 * w:64 * w + 64]
                return qkT[32 * (h - 4):32 * (h - 4) + 32,
                           256 + 64 * w:256 + 64 * w + 64]

            def kx_of(i, w, h):
                qkT = st[("qkT", i)]
                if h < 4:
                    return qkT[32 * h:32 * h + 32,
                               128 + 64 * w:128 + 64 * w + 64]
                return qkT[32 * (h - 4):32 * (h - 4) + 32,
                           384 + 64 * w:384 + 64 * w + 64]

            def back1(i):
                # stage 1 logits (baseline bank=h%4 placement) + exp + bias
                for w in range(2):
                    for h in range(6):
                        b = 32 * (h % 4)
                        o_ap = (pa[64 * w:64 * w + 64, h, 0:16] if h < 4
                                else pa[64 * w:64 * w + 64, h - 4, 16:32])
                        nc.tensor.matmul(
                            o_ap, lhsT=kx_of(i, w, h), rhs=ax_of(i, w, h),
                            start=True, stop=True,
                            tile_position=(b, 64 * w))
                e1x = wpool.tile([128, 6, 16], f32, tag="e1x")
                nc.scalar.activation(
                    out=e1x[:, 0:4, :], in_=pa[:, 0:4, 0:16], func=AF.Exp)
                nc.scalar.activation(
                    out=e1x[:, 4:6, :], in_=pa[:, 0:2, 16:32], func=AF.Exp)
                e1 = wpool.tile([128, 6, 16], f32, tag="e1")
                nc.gpsimd.tensor_mul(
                    out=e1, in0=e1x,
                    in1=eb1_t.rearrange("p (h a) -> p h a", a=16))
                st[("e1", i)] = e1

            def back2a(i):
                e1 = st.pop(("e1", i))
                v2 = st.pop(("v2", i))
                for w in range(2):
                    for h in range(6):
                        nc.tensor.matmul(
                            pa[32 * w:32 * w + 16, 2 + w,
                               256 + 33 * h:256 + 33 * h + 33],
                            lhsT=e1[64 * w:64 * w + 64, h, :],
                            rhs=v2[64 * w:64 * w + 64, h, :],
                            start=True, stop=True,
                            tile_position=(64 * w, 32 * w))
                    for h in range(6):
                        b = 32 * (h % 4)
                        o_ap = (pa[32 * w:32 * w + 16, h, 128:192]
                                if h < 4 else
                                pa[32 * w:32 * w + 16, h - 4, 192:256])
                        nc.tensor.matmul(
                            o_ap, lhsT=ax_of(i, w, h), rhs=qx_of(i, w, h),
                            start=True, stop=True,
                            tile_position=(b, 32 * w))
                x1v = pa[0:48, 2:4, 256:454].rearrange(
                    "p b (h c) -> p b h c", c=33)
                rec1 = spool.tile([48, 2, 6], f32, tag="rec1")
                nc.vector.reciprocal(out=rec1, in_=x1v[:, :, :, 32])
                x1n = wpool.tile([48, 2, 6, 33], f32, tag="x1n")
                nc.vector.tensor_mul(
                    out=x1n, in0=x1v,
                    in1=rec1[:, :, :, None].broadcast_to((48, 2, 6, 33)))
                e2x = wpool.tile([48, 6, 64], f32, tag="e2x")
                nc.scalar.activation(
                    out=e2x[:, 0:4, :], in_=pa[0:48, 0:4, 128:192],
                    func=AF.Exp)
                nc.scalar.activation(
                    out=e2x[:, 4:6, :], in_=pa[0:48, 0:2, 192:256],
                    func=AF.Exp)
                e2 = wpool.tile([48, 6, 64], f32, tag="e2")
                nc.vector.tensor_mul(
                    out=e2, in0=e2x,
                    in1=eb2_t.rearrange("p (h t) -> p h t", t=64))
                for w in range(2):
                    for h in range(6):
                        nc.tensor.matmul(
                            oall[64 * w:64 * w + 64, w, 33 * h:33 * h + 33],
                            lhsT=e2[32 * w:32 * w + 16, h, :],
                            rhs=x1n[32 * w:32 * w + 16, w, h, :],
                            start=True, stop=True,
                            tile_position=(32 * w, 64 * w))

            def back2b(i):
                q, c0, sub = chunks[i]
                wr = 2 * q + sub
                st.pop(("qkT", i)); st.pop(("aTc", i)); st.pop(("qk6", i))
                rec2 = spool.tile([128, 6], f32, tag="rec2")
                osb = iopool.tile([128, 6, 32], f32, tag="osb")
                for w in range(2):
                    ov = oall[64 * w:64 * w + 64, w, 0:198].rearrange(
                        "p (h c) -> p h c", c=33)
                    nc.vector.reciprocal(
                        out=rec2[64 * w:64 * w + 64, :], in_=ov[:, :, 32])
                    nc.vector.tensor_mul(
                        out=osb[64 * w:64 * w + 64], in0=ov[:, :, 0:32],
                        in1=rec2[64 * w:64 * w + 64, :, None
                                 ].broadcast_to((64, 6, 32)))
                for w in range(2):
                    nc.sync.dma_start(
                        out=out_d[wr, :, c0 + w, :, :],
                        in_=osb[64 * w:64 * w + 64].rearrange(
                            "p h c -> p (h c)"))

            # ---- software-pipelined emission, front 2 chunks deep:
            # chunk i+1's transposes sit between MM1_i and MM2_i in the
            # PE stream (filling the exp1 stall), and its norm chain ran
            # during iteration i-1 ----
            n = len(chunks)
            front_pre(0)
            front_mid(0)
            front_pe(0)
            if n > 1:
                front_pre(1)
                front_mid(1)
            for i in range(n):
                back1(i)
                if i + 1 < n:
                    front_pe(i + 1)
                if i + 2 < n:
                    front_pre(i + 2)
                    front_mid(i + 2)
                back2a(i)
                back2b(i)
    if not nc.is_finalized():
        nc.finalize()
    return nc


def _get_nc():
    if "nc" not in _NC_CACHE:
        _NC_CACHE["nc"] = _build_nc()
    return _NC_CACHE["nc"]


def _host_consts(table, i_a2w, i_w2a, ls1, ls2, w11, b11, w12, w21, b21, w22):
    def cpb_table(w1, b1, w2):
        hid = np.maximum(table.reshape(-1, 2) @ w1 + b1, 0.0)
        return hid @ w2  # (121, NH)

    def sigm(x):
        return 1.0 / (1.0 + np.exp(-x))

    bt1 = cpb_table(w11, b11, w12)
    bt2 = cpb_table(w21, b21, w22)
    # stage1 bias: (NH, 16, 64); stage2: (NH, 64, 16)
    b1 = 16.0 * sigm(bt1[i_a2w.reshape(-1)].reshape(16, 64, NH)).transpose(2, 0, 1)
    b2 = 16.0 * sigm(bt2[i_w2a.reshape(-1)].reshape(64, 16, NH)).transpose(2, 0, 1)
    # EB1[t, h, a] = exp(b1[h, a, t]); replicated for the 2-window partition dim
    eb1 = np.exp(b1).transpose(2, 0, 1).reshape(64, 96)
    eb1 = np.tile(eb1, (2, 1)).astype(np.float32)
    # EB2T[a, h, t] = exp(b2[h, t, a])
    eb2t = np.exp(b2).transpose(2, 0, 1).reshape(16, 384).astype(np.float32)
    eb2t = np.tile(eb2t, (3, 1))
    s1 = np.exp(np.minimum(ls1, math.log(100.0))).reshape(NH)
    s2 = np.exp(np.minimum(ls2, math.log(100.0))).reshape(NH)
    scl = np.tile(np.concatenate([s2, s1]).astype(np.float32), (128, 1))
    return eb1, eb2t, np.ascontiguousarray(scl)


def kernel(**inputs):
    kwargs = inputs
    from concourse.bass_utils import run_bass_kernel_spmd

    qkv = np.ascontiguousarray(np.asarray(inputs["qkv"], dtype=np.float32))
    anchor = np.ascontiguousarray(np.asarray(inputs["anchor"], dtype=np.float32))
    table = np.asarray(inputs["table"], dtype=np.float32)
    i_a2w = np.asarray(inputs["index_a2w"]).astype(np.int64)
    i_w2a = np.asarray(inputs["index_w2a"]).astype(np.int64)
    eb1, eb2t, scl = _host_consts(
        table, i_a2w, i_w2a,
        np.asarray(inputs["logit_scale1"], np.float32),
        np.asarray(inputs["logit_scale2"], np.float32),
        np.asarray(inputs["cpb1_w1"], np.float32),
        np.asarray(inputs["cpb1_b1"], np.float32),
        np.asarray(inputs["cpb1_w2"], np.float32),
        np.asarray(inputs["cpb2_w1"], np.float32),
        np.asarray(inputs["cpb2_b1"], np.float32),
        np.asarray(inputs["cpb2_w2"], np.float32),
    )

    in_maps = []
    for c in range(NCORES):
        b = c // 4
        rb = c % 4
        qkv_sh = qkv[b, rb * TOK:(rb + 1) * TOK].reshape(NWR, WS, NWC, WS, C)
        anc_sh = anchor[b, rb * 32:(rb + 1) * 32].reshape(NWR, AWS, NWC, AWS, CO)
        in_maps.append({
            "qkv": np.ascontiguousarray(qkv_sh),
            "anc": np.ascontiguousarray(anc_sh),
            "eb1": eb1, "eb2t": eb2t, "scl": scl,
        })

    nc = _get_nc()
    trace = bool(kwargs.get("_trace"))
    tkw = {}
    if trace:
        tkw = dict(trace=True, tmpdir=kwargs.get("_tmpdir"))
    res = run_bass_kernel_spmd(nc, in_maps, list(range(NCORES)), **tkw)
    results = res.results if hasattr(res, "results") else res
    if trace:
        kernel._last_profile = res

    out = np.empty((B, H * WID, CO), dtype=np.float32)
    for c in range(NCORES):
        b = c // 4
        rb = c % 4
        out[b, rb * TOK:(rb + 1) * TOK] = np.asarray(
            results[c]["out"], dtype=np.float32).reshape(TOK, CO)
    return out


# revision 22
# speedup vs baseline: 1.0089x; 1.0089x over previous
"""AnchorStripeAttention Trainium2 kernel (8 NeuronCores, data-parallel).

v2: engine-rebalanced rewrite of the baseline.
  - Shard over batch (2) x window-row-blocks (4): core c -> batch c//4,
    pixel rows [ (c%4)*64, (c%4+1)*64 ) -> 16384 contiguous tokens of qkv,
    32 contiguous rows of the 128-row anchor image. 256 windows per core.
  - CPB-MLP bias tables + logit scales precomputed on host (params only).
  - Anchors: loaded once [128, 32, 192] (partition = wr*16 + i*4 + j),
    l2-normalized densely ONCE per core (baseline redid this per chunk at
    16/128 occupancy).
  - Per 2-window chunk: qk squares, q/k scale-mul and e1-mul on GpSimd
    (idle in baseline); reductions/recips/psum-evac on Vector; exps+sqrt
    on Scalar; matmuls ordered so consecutive PE instructions in
    different array tiles always hit different PSUM banks.
  PSUM map (8 banks):
    pa[128,4,512] banks0-3: attn1T (b0,b1 cols0:96, bank=w),
      attn2 (b0-3 cols128:256, bank=h%4 as baseline),
      x1e (b2,b3 cols256:454, bank=2+w)
    oall[128,2,512] banks4-5 (bank=4+w)
    tpA[128,512] bank6 (q/k transposes), tpB[128,64] bank7 (anc transposes)
  Softmax max-subtraction skipped (logits bounded ~26); masks all-zero.
"""

import math
import numpy as np

B = 2
H = 256
WID = 256
NH = 6
DH = 32
CO = NH * DH
C = 3 * CO
WS = 8
AWS = 4
T = 121
NCORES = 8
NWR = 8         # window-rows per core
NWC = 32        # window-cols
TOK = 16384     # tokens per core shard

_NC_CACHE = {}


def _build_nc():
    import concourse.bass as bass
    from concourse import bacc
    import concourse.mybir as mybir
    from concourse.tile import TileContext
    from concourse.masks import make_identity

    f32 = mybir.dt.float32
    AX = mybir.AxisListType
    OP = mybir.AluOpType
    AF = mybir.ActivationFunctionType

    nc = bacc.Bacc("TRN2")
    qkv_d = nc.declare_dram_parameter("qkv", [NWR, WS, NWC, WS, C], f32, isOutput=False)
    anc_d = nc.declare_dram_parameter("anc", [NWR, AWS, NWC, AWS, CO], f32, isOutput=False)
    eb1_d = nc.declare_dram_parameter("eb1", [128, 96], f32, isOutput=False)
    eb2_d = nc.declare_dram_parameter("eb2t", [48, 384], f32, isOutput=False)
    scl_d = nc.declare_dram_parameter("scl", [128, 12], f32, isOutput=False)
    out_d = nc.declare_dram_parameter("out", [NWR, WS, NWC, WS, CO], f32, isOutput=True)

    with TileContext(nc) as tc:
        with (
            tc.tile_pool(name="const", bufs=1) as cpool,
            tc.tile_pool(name="io", bufs=3) as iopool,
            tc.tile_pool(name="work", bufs=2) as wpool,
            tc.tile_pool(name="small", bufs=3) as spool,
            tc.tile_pool(name="ps_pa", bufs=1, space="PSUM") as ps_pa,
            tc.tile_pool(name="ps_o", bufs=1, space="PSUM") as ps_o,
            tc.tile_pool(name="ps_tp", bufs=1, space="PSUM") as ps_tp,
        ):
            ident = cpool.tile([128, 128], f32)
            make_identity(nc, ident)
            eb1_t = cpool.tile([128, 96], f32)
            nc.sync.dma_start(out=eb1_t, in_=eb1_d[:, :])
            eb2_t = cpool.tile([48, 384], f32)
            nc.sync.dma_start(out=eb2_t, in_=eb2_d[:, :])
            scl_t = cpool.tile([128, 12], f32)
            nc.sync.dma_start(out=scl_t, in_=scl_d[:, :])

            # ---- anchors: load all, l2-normalize once, densely ----
            anc_all = cpool.tile([128, NWC, 192], f32)
            for wr in range(NWR):
                for i in range(AWS):
                    nc.sync.dma_start(
                        out=anc_all[16 * wr + 4 * i:16 * wr + 4 * i + 4, :, :],
                        in_=anc_d[wr, i, :, :, :].rearrange("w j c -> j w c"))
            asq = cpool.tile([128, NWC, 192], f32)
            nc.vector.tensor_mul(out=asq, in0=anc_all, in1=anc_all)
            ssA = cpool.tile([128, NWC * 6], f32)
            nc.vector.tensor_reduce(
                out=ssA,
                in_=asq.rearrange("p w (h c) -> p (w h) c", c=32),
                axis=AX.X, op=OP.add)
            rssA = cpool.tile([128, NWC * 6], f32)
            nc.vector.reciprocal(out=rssA, in_=ssA)
            rsA = cpool.tile([128, NWC * 6], f32)
            nc.scalar.sqrt(out=rsA, in_=rssA)
            anc_n = cpool.tile([128, NWC, 192], f32)
            nc.vector.tensor_mul(
                out=anc_n.rearrange("p w (h c) -> p w h c", c=32),
                in0=anc_all.rearrange("p w (h c) -> p w h c", c=32),
                in1=rsA.rearrange("p (w h) -> p w h", h=6)[:, :, :, None
                    ].broadcast_to((128, NWC, 6, 32)))

            # ---- persistent PSUM tiles (8 banks total), zeroed once so
            # the padded single-instruction views read deterministic data
            pa = ps_pa.tile([128, 4, 512], f32, tag="pa")
            nc.vector.memset(pa, 0.0)
            oall = ps_o.tile([128, 2, 512], f32, tag="oall")
            nc.vector.memset(oall, 0.0)
            tpA = ps_tp.tile([128, 512], f32, tag="tpA")
            nc.vector.memset(tpA[64:128, 256:512], 0.0)
            tpB = ps_tp.tile([128, 128], f32, tag="tpB")

            chunks = [(qq, 2 * cc, ss)
                      for qq in range(NWR // 2)
                      for cc in range(NWC // 2)
                      for ss in range(2)]

            st = {}

            def pair_prep(i):
                # anchor staging + transposes for a row-pair; shared by
                # the pair's chunks via the aT tag (bufs must cover reuse)
                q, c0, sub = chunks[i]
                if sub != 0:
                    return st[("aT", q, c0)]
                anc_st = wpool.tile([32, 2, 192], f32, tag="anc_st")
                nc.sync.dma_start(
                    out=anc_st, in_=anc_n[32 * q:32 * q + 32, c0:c0 + 2, :])
                for w in range(2):
                    nc.tensor.transpose(
                        out=tpB[0:128, 32 * w:32 * w + 32],
                        in_=anc_st[0:32, w, 0:128],
                        identity=ident[0:32, 0:32])
                    nc.tensor.transpose(
                        out=tpB[0:64, 64 + 32 * w:64 + 32 * w + 32],
                        in_=anc_st[0:32, w, 128:192],
                        identity=ident[0:32, 0:32])
                aT = wpool.tile([128, 128], f32, tag="aT")
                nc.vector.tensor_copy(out=aT[:, 0:64], in_=tpB[:, 0:64])
                nc.vector.tensor_copy(
                    out=aT[0:64, 64:128], in_=tpB[0:64, 64:128])
                st[("aT", q, c0)] = aT
                return aT

            def front_pre(i):
                q, c0, sub = chunks[i]
                wr = 2 * q + sub
                qk6 = iopool.tile([128, 576], f32, tag="qk6")
                for w in range(2):
                    nc.sync.dma_start(
                        out=qk6[64 * w:64 * w + 64, :],
                        in_=qkv_d[wr, :, c0 + w, :, :])
                v2 = wpool.tile([128, 6, 33], f32, tag="v2")
                nc.gpsimd.memset(v2[:, :, 32:33], 1.0)
                nc.gpsimd.tensor_copy(
                    out=v2[:, :, 0:32],
                    in_=qk6[:, 384:576].rearrange("p (h c) -> p h c", c=32))
                sq = wpool.tile([128, 384], f32, tag="sq")
                nc.gpsimd.tensor_mul(
                    out=sq, in0=qk6[:, 0:384], in1=qk6[:, 0:384])
                st[("qk6", i)] = qk6
                st[("v2", i)] = v2
                st[("sq", i)] = sq

            def front_mid(i):
                qk6 = st[("qk6", i)]
                sq = st.pop(("sq", i))
                ss = spool.tile([128, 12], f32, tag="ss")
                nc.vector.tensor_reduce(
                    out=ss, in_=sq.rearrange("p (h c) -> p h c", c=32),
                    axis=AX.X, op=OP.add)
                rss = spool.tile([128, 12], f32, tag="rss")
                nc.vector.reciprocal(out=rss, in_=ss)
                rs = spool.tile([128, 12], f32, tag="rs")
                nc.scalar.sqrt(out=rs, in_=rss)
                r = spool.tile([128, 12], f32, tag="r")
                nc.vector.tensor_mul(out=r, in0=rs, in1=scl_t)
                qks = wpool.tile([128, 384], f32, tag="qks")
                nc.gpsimd.tensor_mul(
                    out=qks.rearrange("p (h c) -> p h c", c=32),
                    in0=qk6[:, 0:384].rearrange("p (h c) -> p h c", c=32),
                    in1=r[:, :, None].broadcast_to((128, 12, 32)))
                st[("qks", i)] = qks

            def front_pe(i):
                aT = pair_prep(i)
                qks = st.pop(("qks", i))
                nc.tensor.transpose(
                    out=tpA[:, 0:128], in_=qks[:, 0:128], identity=ident)
                nc.tensor.transpose(
                    out=tpA[:, 128:256], in_=qks[:, 192:320], identity=ident)
                nc.tensor.transpose(
                    out=tpA[0:64, 256:384], in_=qks[:, 128:192],
                    identity=ident)
                nc.tensor.transpose(
                    out=tpA[0:64, 384:512], in_=qks[:, 320:384],
                    identity=ident)
                qkT = wpool.tile([128, 512], f32, tag="qkT")
                nc.vector.tensor_copy(out=qkT, in_=tpA)
                st[("qkT", i)] = qkT
                st[("aTc", i)] = aT

            def ax_of(i, w, h):
                q, c0, sub = chunks[i]
                aT = st[("aTc", i)]
                if h < 4:
                    return aT[32 * h:32 * h + 32,
                              32 * w + 16 * sub:32 * w + 16 * sub + 16]
                return aT[32 * (h - 4):32 * (h - 4) + 32,
                          64 + 32 * w + 16 * sub:64 + 32 * w + 16 * sub + 16]

            def qx_of(i, w, h):
                qkT = st[("qkT", i)]
                if h < 4:
                    return qkT[32 * h:32 * h + 32, 64 * w:64 * w + 64]
                return qkT[32 * (h - 4):32 * (h - 4) + 32,
                           256 + 64 * w:256 + 64 * w + 64]

            def kx_of(i, w, h):
                qkT = st[("qkT", i)]
                if h < 4:
                    return qkT[32 * h:32 * h + 32,
                               128 + 64 * w:128 + 64 * w + 64]
                return qkT[32 * (h - 4):32 * (h - 4) + 32,
                           384 + 64 * w:384 + 64 * w + 64]

            def back1(i):
                # stage 1 logits (baseline bank=h%4 placement) + exp + bias
                for w in range(2):
                    for h in range(6):
                        b = 32 * (h % 4)
                        o_ap = (pa[64 * w:64 * w + 64, h, 0:16] if h < 4
                                else pa[64 * w:64 * w + 64, h - 4, 16:32])
                        nc.tensor.matmul(
                            o_ap, lhsT=kx_of(i, w, h), rhs=ax_of(i, w, h),
                            start=True, stop=True,
                            tile_position=(b, 64 * w))
                e1x = wpool.tile([128, 6, 16], f32, tag="e1x")
                nc.scalar.activation(
                    out=e1x[:, 0:4, :], in_=pa[:, 0:4, 0:16], func=AF.Exp)
                nc.scalar.activation(
                    out=e1x[:, 4:6, :], in_=pa[:, 0:2, 16:32], func=AF.Exp)
                e1 = wpool.tile([128, 6, 16], f32, tag="e1")
                nc.gpsimd.tensor_mul(
                    out=e1, in0=e1x,
                    in1=eb1_t.rearrange("p (h a) -> p h a", a=16))
                st[("e1", i)] = e1

            def back2a(i):
                e1 = st.pop(("e1", i))
                v2 = st.pop(("v2", i))
                for w in range(2):
                    for h in range(6):
                        nc.tensor.matmul(
                            pa[32 * w:32 * w + 16, 2 + w,
                               256 + 33 * h:256 + 33 * h + 33],
                            lhsT=e1[64 * w:64 * w + 64, h, :],
                            rhs=v2[64 * w:64 * w + 64, h, :],
                            start=True, stop=True,
                            tile_position=(64 * w, 32 * w))
                    for h in range(6):
                        b = 32 * (h % 4)
                        o_ap = (pa[32 * w:32 * w + 16, h, 128:192]
                                if h < 4 else
                                pa[32 * w:32 * w + 16, h - 4, 192:256])
                        nc.tensor.matmul(
                            o_ap, lhsT=ax_of(i, w, h), rhs=qx_of(i, w, h),
                            start=True, stop=True,
                            tile_position=(b, 32 * w))
                x1v = pa[0:48, 2:4, 256:454].rearrange(
                    "p b (h c) -> p b h c", c=33)
                rec1 = spool.tile([48, 2, 6], f32, tag="rec1")
                nc.vector.reciprocal(out=rec1, in_=x1v[:, :, :, 32])
                x1n = wpool.tile([48, 2, 6, 33], f32, tag="x1n")
                nc.vector.tensor_mul(
                    out=x1n, in0=x1v,
                    in1=rec1[:, :, :, None].broadcast_to((48, 2, 6, 33)))
                e2x = wpool.tile([48, 6, 64], f32, tag="e2x")
                nc.scalar.activation(
                    out=e2x[:, 0:4, :], in_=pa[0:48, 0:4, 128:192],
                    func=AF.Exp)
                nc.scalar.activation(
                    out=e2x[:, 4:6, :], in_=pa[0:48, 0:2, 192:256],
                    func=AF.Exp)
                e2 = wpool.tile([48, 6, 64], f32, tag="e2")
                nc.vector.tensor_mul(
                    out=e2, in0=e2x,
                    in1=eb2_t.rearrange("p (h t) -> p h t", t=64))
                for w in range(2):
                    for h in range(6):
                        nc.tensor.matmul(
                            oall[64 * w:64 * w + 64, w, 33 * h:33 * h + 33],
                            lhsT=e2[32 * w:32 * w + 16, h, :],
                            rhs=x1n[32 * w:32 * w + 16, w, h, :],
                            start=True, stop=True,
                            tile_position=(32 * w, 64 * w))

            def back2b(i):
                q, c0, sub = chunks[i]
                wr = 2 * q + sub
                st.pop(("qkT", i)); st.pop(("aTc", i)); st.pop(("qk6", i))
                rec2 = spool.tile([128, 6], f32, tag="rec2")
                osb = iopool.tile([128, 6, 32], f32, tag="osb")
                for w in range(2):
                    ov = oall[64 * w:64 * w + 64, w, 0:198].rearrange(
                        "p (h c) -> p h c", c=33)
                    nc.vector.reciprocal(
                        out=rec2[64 * w:64 * w + 64, :], in_=ov[:, :, 32])
                    nc.vector.tensor_mul(
                        out=osb[64 * w:64 * w + 64], in0=ov[:, :, 0:32],
                        in1=rec2[64 * w:64 * w + 64, :, None
                                 ].broadcast_to((64, 6, 32)))
                for w in range(2):
                    nc.sync.dma_start(
                        out=out_d[wr, :, c0 + w, :, :],
                        in_=osb[64 * w:64 * w + 64].rearrange(
                            "p h c -> p (h c)"))

            # ---- software-pipelined emission: chunk i+1's norm/transpose
            # front runs in the gaps of chunk i's matmul/exp back half ----
            n = len(chunks)
            front_pre(0)
            front_mid(0)
            front_pe(0)
            for i in range(n):
                back1(i)
                if i + 1 < n:
                    front_pre(i + 1)
                    front_mid(i + 1)
                back2a(i)
                if i + 1 < n:
                    front_pe(i + 1)
                back2b(i)
    if not nc.is_finalized():
        nc.finalize()
    return nc


def _get_nc():
    if "nc" not in _NC_CACHE:
        _NC_CACHE["nc"] = _build_nc()
    return _NC_CACHE["nc"]


def _host_consts(table, i_a2w, i_w2a, ls1, ls2, w11, b11, w12, w21, b21, w22):
    def cpb_table(w1, b1, w2):
        hid = np.maximum(table.reshape(-1, 2) @ w1 + b1, 0.0)
        return hid @ w2  # (121, NH)

    def sigm(x):
        return 1.0 / (1.0 + np.exp(-x))

    bt1 = cpb_table(w11, b11, w12)
    bt2 = cpb_table(w21, b21, w22)
    # stage1 bias: (NH, 16, 64); stage2: (NH, 64, 16)
    b1 = 16.0 * sigm(bt1[i_a2w.reshape(-1)].reshape(16, 64, NH)).transpose(2, 0, 1)
    b2 = 16.0 * sigm(bt2[i_w2a.reshape(-1)].reshape(64, 16, NH)).transpose(2, 0, 1)
    # EB1[t, h, a] = exp(b1[h, a, t]); replicated for the 2-window partition dim
    eb1 = np.exp(b1).transpose(2, 0, 1).reshape(64, 96)
    eb1 = np.tile(eb1, (2, 1)).astype(np.float32)
    # EB2T[a, h, t] = exp(b2[h, t, a])
    eb2t = np.exp(b2).transpose(2, 0, 1).reshape(16, 384).astype(np.float32)
    eb2t = np.tile(eb2t, (3, 1))
    s1 = np.exp(np.minimum(ls1, math.log(100.0))).reshape(NH)
    s2 = np.exp(np.minimum(ls2, math.log(100.0))).reshape(NH)
    scl = np.tile(np.concatenate([s2, s1]).astype(np.float32), (128, 1))
    return eb1, eb2t, np.ascontiguousarray(scl)


def kernel(**inputs):
    kwargs = inputs
    from concourse.bass_utils import run_bass_kernel_spmd

    qkv = np.ascontiguousarray(np.asarray(inputs["qkv"], dtype=np.float32))
    anchor = np.ascontiguousarray(np.asarray(inputs["anchor"], dtype=np.float32))
    table = np.asarray(inputs["table"], dtype=np.float32)
    i_a2w = np.asarray(inputs["index_a2w"]).astype(np.int64)
    i_w2a = np.asarray(inputs["index_w2a"]).astype(np.int64)
    eb1, eb2t, scl = _host_consts(
        table, i_a2w, i_w2a,
        np.asarray(inputs["logit_scale1"], np.float32),
        np.asarray(inputs["logit_scale2"], np.float32),
        np.asarray(inputs["cpb1_w1"], np.float32),
        np.asarray(inputs["cpb1_b1"], np.float32),
        np.asarray(inputs["cpb1_w2"], np.float32),
        np.asarray(inputs["cpb2_w1"], np.float32),
        np.asarray(inputs["cpb2_b1"], np.float32),
        np.asarray(inputs["cpb2_w2"], np.float32),
    )

    in_maps = []
    for c in range(NCORES):
        b = c // 4
        rb = c % 4
        qkv_sh = qkv[b, rb * TOK:(rb + 1) * TOK].reshape(NWR, WS, NWC, WS, C)
        anc_sh = anchor[b, rb * 32:(rb + 1) * 32].reshape(NWR, AWS, NWC, AWS, CO)
        in_maps.append({
            "qkv": np.ascontiguousarray(qkv_sh),
            "anc": np.ascontiguousarray(anc_sh),
            "eb1": eb1, "eb2t": eb2t, "scl": scl,
        })

    nc = _get_nc()
    trace = bool(kwargs.get("_trace"))
    tkw = {}
    if trace:
        tkw = dict(trace=True, tmpdir=kwargs.get("_tmpdir"))
    res = run_bass_kernel_spmd(nc, in_maps, list(range(NCORES)), **tkw)
    results = res.results if hasattr(res, "results") else res
    if trace:
        kernel._last_profile = res

    out = np.empty((B, H * WID, CO), dtype=np.float32)
    for c in range(NCORES):
        b = c // 4
        rb = c % 4
        out[b, rb * TOK:(rb + 1) * TOK] = np.asarray(
            results[c]["out"], dtype=np.float32).reshape(TOK, CO)
    return out


# revision 24
# speedup vs baseline: 1.0093x; 1.0004x over previous
"""AnchorStripeAttention Trainium2 kernel (8 NeuronCores, data-parallel).

v2: engine-rebalanced rewrite of the baseline.
  - Shard over batch (2) x window-row-blocks (4): core c -> batch c//4,
    pixel rows [ (c%4)*64, (c%4+1)*64 ) -> 16384 contiguous tokens of qkv,
    32 contiguous rows of the 128-row anchor image. 256 windows per core.
  - CPB-MLP bias tables + logit scales precomputed on host (params only).
  - Anchors: loaded once [128, 32, 192] (partition = wr*16 + i*4 + j),
    l2-normalized densely ONCE per core (baseline redid this per chunk at
    16/128 occupancy).
  - Per 2-window chunk: qk squares, q/k scale-mul and e1-mul on GpSimd
    (idle in baseline); reductions/recips/psum-evac on Vector; exps+sqrt
    on Scalar; matmuls ordered so consecutive PE instructions in
    different array tiles always hit different PSUM banks.
  PSUM map (8 banks):
    pa[128,4,512] banks0-3: attn1T (b0,b1 cols0:96, bank=w),
      attn2 (b0-3 cols128:256, bank=h%4 as baseline),
      x1e (b2,b3 cols256:454, bank=2+w)
    oall[128,2,512] banks4-5 (bank=4+w)
    tpA[128,512] bank6 (q/k transposes), tpB[128,64] bank7 (anc transposes)
  Softmax max-subtraction skipped (logits bounded ~26); masks all-zero.
"""

import math
import numpy as np

B = 2
H = 256
WID = 256
NH = 6
DH = 32
CO = NH * DH
C = 3 * CO
WS = 8
AWS = 4
T = 121
NCORES = 8
NWR = 8         # window-rows per core
NWC = 32        # window-cols
TOK = 16384     # tokens per core shard

_NC_CACHE = {}


def _build_nc():
    import concourse.bass as bass
    from concourse import bacc
    import concourse.mybir as mybir
    from concourse.tile import TileContext
    from concourse.masks import make_identity

    f32 = mybir.dt.float32
    AX = mybir.AxisListType
    OP = mybir.AluOpType
    AF = mybir.ActivationFunctionType

    nc = bacc.Bacc("TRN2")
    qkv_d = nc.declare_dram_parameter("qkv", [NWR, WS, NWC, WS, C], f32, isOutput=False)
    anc_d = nc.declare_dram_parameter("anc", [NWR, AWS, NWC, AWS, CO], f32, isOutput=False)
    eb1_d = nc.declare_dram_parameter("eb1", [128, 96], f32, isOutput=False)
    eb2_d = nc.declare_dram_parameter("eb2t", [48, 384], f32, isOutput=False)
    scl_d = nc.declare_dram_parameter("scl", [128, 12], f32, isOutput=False)
    out_d = nc.declare_dram_parameter("out", [NWR, WS, NWC, WS, CO], f32, isOutput=True)

    with TileContext(nc) as tc:
        with (
            tc.tile_pool(name="const", bufs=1) as cpool,
            tc.tile_pool(name="io", bufs=3) as iopool,
            tc.tile_pool(name="work", bufs=2) as wpool,
            tc.tile_pool(name="small", bufs=3) as spool,
            tc.tile_pool(name="ps_pa", bufs=1, space="PSUM") as ps_pa,
            tc.tile_pool(name="ps_o", bufs=1, space="PSUM") as ps_o,
            tc.tile_pool(name="ps_tp", bufs=1, space="PSUM") as ps_tp,
        ):
            ident = cpool.tile([128, 128], f32)
            make_identity(nc, ident)
            eb1_t = cpool.tile([128, 96], f32)
            nc.sync.dma_start(out=eb1_t, in_=eb1_d[:, :])
            eb2_t = cpool.tile([48, 384], f32)
            nc.sync.dma_start(out=eb2_t, in_=eb2_d[:, :])
            scl_t = cpool.tile([128, 12], f32)
            nc.sync.dma_start(out=scl_t, in_=scl_d[:, :])

            # ---- anchors: load all, l2-normalize once, densely ----
            anc_all = cpool.tile([128, NWC, 192], f32)
            for wr in range(NWR):
                for i in range(AWS):
                    nc.sync.dma_start(
                        out=anc_all[16 * wr + 4 * i:16 * wr + 4 * i + 4, :, :],
                        in_=anc_d[wr, i, :, :, :].rearrange("w j c -> j w c"))
            asq = cpool.tile([128, NWC, 192], f32)
            nc.vector.tensor_mul(out=asq, in0=anc_all, in1=anc_all)
            ssA = cpool.tile([128, NWC * 6], f32)
            nc.vector.tensor_reduce(
                out=ssA,
                in_=asq.rearrange("p w (h c) -> p (w h) c", c=32),
                axis=AX.X, op=OP.add)
            rssA = cpool.tile([128, NWC * 6], f32)
            nc.vector.reciprocal(out=rssA, in_=ssA)
            rsA = cpool.tile([128, NWC * 6], f32)
            nc.scalar.sqrt(out=rsA, in_=rssA)
            anc_n = cpool.tile([128, NWC, 192], f32)
            nc.vector.tensor_mul(
                out=anc_n.rearrange("p w (h c) -> p w h c", c=32),
                in0=anc_all.rearrange("p w (h c) -> p w h c", c=32),
                in1=rsA.rearrange("p (w h) -> p w h", h=6)[:, :, :, None
                    ].broadcast_to((128, NWC, 6, 32)))

            # ---- persistent PSUM tiles (8 banks total), zeroed once so
            # the padded single-instruction views read deterministic data
            pa = ps_pa.tile([128, 4, 512], f32, tag="pa")
            nc.vector.memset(pa, 0.0)
            oall = ps_o.tile([128, 2, 512], f32, tag="oall")
            nc.vector.memset(oall, 0.0)
            tpA = ps_tp.tile([128, 512], f32, tag="tpA")
            nc.vector.memset(tpA[64:128, 256:512], 0.0)
            tpB = ps_tp.tile([128, 128], f32, tag="tpB")

            chunks = [(qq, 2 * cc, ss)
                      for qq in range(NWR // 2)
                      for cc in range(NWC // 2)
                      for ss in range(2)]

            st = {}

            def pair_prep(i):
                # anchor staging + transposes for a row-pair; shared by
                # the pair's chunks via the aT tag (bufs must cover reuse)
                q, c0, sub = chunks[i]
                if sub != 0:
                    return st[("aT", q, c0)]
                anc_st = wpool.tile([32, 2, 192], f32, tag="anc_st")
                nc.sync.dma_start(
                    out=anc_st, in_=anc_n[32 * q:32 * q + 32, c0:c0 + 2, :])
                for w in range(2):
                    nc.tensor.transpose(
                        out=tpB[0:128, 32 * w:32 * w + 32],
                        in_=anc_st[0:32, w, 0:128],
                        identity=ident[0:32, 0:32])
                    nc.tensor.transpose(
                        out=tpB[0:64, 64 + 32 * w:64 + 32 * w + 32],
                        in_=anc_st[0:32, w, 128:192],
                        identity=ident[0:32, 0:32])
                aT = wpool.tile([128, 128], f32, tag="aT")
                nc.vector.tensor_copy(out=aT[:, 0:64], in_=tpB[:, 0:64])
                nc.vector.tensor_copy(
                    out=aT[0:64, 64:128], in_=tpB[0:64, 64:128])
                st[("aT", q, c0)] = aT
                return aT

            def front_pre(i):
                q, c0, sub = chunks[i]
                wr = 2 * q + sub
                qk6 = iopool.tile([128, 576], f32, tag="qk6")
                for w in range(2):
                    nc.sync.dma_start(
                        out=qk6[64 * w:64 * w + 64, :],
                        in_=qkv_d[wr, :, c0 + w, :, :])
                v2 = wpool.tile([128, 6, 33], f32, tag="v2")
                nc.gpsimd.memset(v2[:, :, 32:33], 1.0)
                nc.gpsimd.tensor_copy(
                    out=v2[:, :, 0:32],
                    in_=qk6[:, 384:576].rearrange("p (h c) -> p h c", c=32))
                sq = wpool.tile([128, 384], f32, tag="sq")
                nc.gpsimd.tensor_mul(
                    out=sq, in0=qk6[:, 0:384], in1=qk6[:, 0:384])
                st[("qk6", i)] = qk6
                st[("v2", i)] = v2
                st[("sq", i)] = sq

            def front_mid(i):
                qk6 = st[("qk6", i)]
                sq = st.pop(("sq", i))
                ss = spool.tile([128, 12], f32, tag="ss")
                nc.vector.tensor_reduce(
                    out=ss, in_=sq.rearrange("p (h c) -> p h c", c=32),
                    axis=AX.X, op=OP.add)
                rss = spool.tile([128, 12], f32, tag="rss")
                nc.vector.reciprocal(out=rss, in_=ss)
                rs = spool.tile([128, 12], f32, tag="rs")
                nc.scalar.sqrt(out=rs, in_=rss)
                r = spool.tile([128, 12], f32, tag="r")
                nc.vector.tensor_mul(out=r, in0=rs, in1=scl_t)
                qks = wpool.tile([128, 384], f32, tag="qks")
                nc.gpsimd.tensor_mul(
                    out=qks.rearrange("p (h c) -> p h c", c=32),
                    in0=qk6[:, 0:384].rearrange("p (h c) -> p h c", c=32),
                    in1=r[:, :, None].broadcast_to((128, 12, 32)))
                st[("qks", i)] = qks

            def front_pe(i):
                aT = pair_prep(i)
                qks = st.pop(("qks", i))
                nc.tensor.transpose(
                    out=tpA[:, 0:128], in_=qks[:, 0:128], identity=ident)
                nc.tensor.transpose(
                    out=tpA[:, 128:256], in_=qks[:, 192:320], identity=ident)
                nc.tensor.transpose(
                    out=tpA[0:64, 256:384], in_=qks[:, 128:192],
                    identity=ident)
                nc.tensor.transpose(
                    out=tpA[0:64, 384:512], in_=qks[:, 320:384],
                    identity=ident)
                qkT = wpool.tile([128, 512], f32, tag="qkT")
                nc.vector.tensor_copy(out=qkT, in_=tpA)
                st[("qkT", i)] = qkT
                st[("aTc", i)] = aT

            def ax_of(i, w, h):
                q, c0, sub = chunks[i]
                aT = st[("aTc", i)]
                if h < 4:
                    return aT[32 * h:32 * h + 32,
                              32 * w + 16 * sub:32 * w + 16 * sub + 16]
                return aT[32 * (h - 4):32 * (h - 4) + 32,
                          64 + 32 * w + 16 * sub:64 + 32 * w + 16 * sub + 16]

            def qx_of(i, w, h):
                qkT = st[("qkT", i)]
                if h < 4:
                    return qkT[32 * h:32 * h + 32, 64 * w:64 * w + 64]
                return qkT[32 * (h - 4):32 * (h - 4) + 32,
                           256 + 64 * w:256 + 64 * w + 64]

            def kx_of(i, w, h):
                qkT = st[("qkT", i)]
                if h < 4:
                    return qkT[32 * h:32 * h + 32,
                               128 + 64 * w:128 + 64 * w + 64]
                return qkT[32 * (h - 4):32 * (h - 4) + 32,
                           384 + 64 * w:384 + 64 * w + 64]

            def back1(i):
                # stage 1 logits (baseline bank=h%4 placement) + exp + bias
                for w in range(2):
                    for h in range(6):
                        b = 32 * (h % 4)
                        o_ap = (pa[64 * w:64 * w + 64, h, 0:16] if h < 4
                                else pa[64 * w:64 * w + 64, h - 4, 16:32])
                        nc.tensor.matmul(
                            o_ap, lhsT=kx_of(i, w, h), rhs=ax_of(i, w, h),
                            start=True, stop=True,
                            tile_position=(b, 64 * w))
                e1x = wpool.tile([128, 6, 16], f32, tag="e1x")
                nc.scalar.activation(
                    out=e1x[:, 0:4, :], in_=pa[:, 0:4, 0:16], func=AF.Exp)
                nc.scalar.activation(
                    out=e1x[:, 4:6, :], in_=pa[:, 0:2, 16:32], func=AF.Exp)
                e1 = wpool.tile([128, 6, 16], f32, tag="e1")
                nc.gpsimd.tensor_mul(
                    out=e1, in0=e1x,
                    in1=eb1_t.rearrange("p (h a) -> p h a", a=16))
                st[("e1", i)] = e1

            def back2a(i):
                e1 = st.pop(("e1", i))
                v2 = st.pop(("v2", i))
                for w in range(2):
                    for h in range(6):
                        nc.tensor.matmul(
                            pa[32 * w:32 * w + 16, 2 + w,
                               256 + 33 * h:256 + 33 * h + 33],
                            lhsT=e1[64 * w:64 * w + 64, h, :],
                            rhs=v2[64 * w:64 * w + 64, h, :],
                            start=True, stop=True,
                            tile_position=(64 * w, 32 * w))
                    for h in range(6):
                        b = 32 * (h % 4)
                        o_ap = (pa[32 * w:32 * w + 16, h, 128:192]
                                if h < 4 else
                                pa[32 * w:32 * w + 16, h - 4, 192:256])
                        nc.tensor.matmul(
                            o_ap, lhsT=ax_of(i, w, h), rhs=qx_of(i, w, h),
                            start=True, stop=True,
                            tile_position=(b, 32 * w))
                x1v = pa[0:48, 2:4, 256:454].rearrange(
                    "p b (h c) -> p b h c", c=33)
                rec1 = spool.tile([48, 2, 6], f32, tag="rec1")
                nc.vector.reciprocal(out=rec1, in_=x1v[:, :, :, 32])
                x1n = wpool.tile([48, 2, 6, 33], f32, tag="x1n")
                nc.vector.tensor_mul(
                    out=x1n, in0=x1v,
                    in1=rec1[:, :, :, None].broadcast_to((48, 2, 6, 33)))
                e2x = wpool.tile([48, 6, 64], f32, tag="e2x")
                nc.scalar.activation(
                    out=e2x[:, 0:4, :], in_=pa[0:48, 0:4, 128:192],
                    func=AF.Exp)
                nc.scalar.activation(
                    out=e2x[:, 4:6, :], in_=pa[0:48, 0:2, 192:256],
                    func=AF.Exp)
                e2 = wpool.tile([48, 6, 64], f32, tag="e2")
                nc.vector.tensor_mul(
                    out=e2, in0=e2x,
                    in1=eb2_t.rearrange("p (h t) -> p h t", t=64))
                for w in range(2):
                    for h in range(6):
                        nc.tensor.matmul(
                            oall[64 * w:64 * w + 64, w, 33 * h:33 * h + 33],
                            lhsT=e2[32 * w:32 * w + 16, h, :],
                            rhs=x1n[32 * w:32 * w + 16, w, h, :],
                            start=True, stop=True,
                            tile_position=(32 * w, 64 * w))

            def back2b(i):
                q, c0, sub = chunks[i]
                wr = 2 * q + sub
                st.pop(("qkT", i)); st.pop(("aTc", i)); st.pop(("qk6", i))
                rec2 = spool.tile([128, 6], f32, tag="rec2")
                osb = iopool.tile([128, 6, 32], f32, tag="osb")
                for w in range(2):
                    ov = oall[64 * w:64 * w + 64, w, 0:198].rearrange(
                        "p (h c) -> p h c", c=33)
                    nc.vector.reciprocal(
                        out=rec2[64 * w:64 * w + 64, :], in_=ov[:, :, 32])
                    nc.vector.tensor_mul(
                        out=osb[64 * w:64 * w + 64], in0=ov[:, :, 0:32],
                        in1=rec2[64 * w:64 * w + 64, :, None
                                 ].broadcast_to((64, 6, 32)))
                for w in range(2):
                    nc.sync.dma_start(
                        out=out_d[wr, :, c0 + w, :, :],
                        in_=osb[64 * w:64 * w + 64].rearrange(
                            "p h c -> p (h c)"))

            # ---- software-pipelined emission: chunk i+1's norm/transpose
            # front runs in the gaps of chunk i's matmul/exp back half ----
            n = len(chunks)
            front_pre(0)
            front_mid(0)
            front_pe(0)
            for i in range(n):
                back1(i)
                if i + 1 < n:
                    front_pre(i + 1)
                    front_mid(i + 1)
                back2a(i)
                if i + 1 < n:
                    front_pe(i + 1)
                back2b(i)
    if not nc.is_finalized():
        nc.finalize()
    return nc


def _get_nc():
    if "nc" not in _NC_CACHE:
        _NC_CACHE["nc"] = _build_nc()
    return _NC_CACHE["nc"]


def _host_consts(table, i_a2w, i_w2a, ls1, ls2, w11, b11, w12, w21, b21, w22):
    def cpb_table(w1, b1, w2):
        hid = np.maximum(table.reshape(-1, 2) @ w1 + b1, 0.0)
        return hid @ w2  # (121, NH)

    def sigm(x):
        return 1.0 / (1.0 + np.exp(-x))

    bt1 = cpb_table(w11, b11, w12)
    bt2 = cpb_table(w21, b21, w22)
    # stage1 bias: (NH, 16, 64); stage2: (NH, 64, 16)
    b1 = 16.0 * sigm(bt1[i_a2w.reshape(-1)].reshape(16, 64, NH)).transpose(2, 0, 1)
    b2 = 16.0 * sigm(bt2[i_w2a.reshape(-1)].reshape(64, 16, NH)).transpose(2, 0, 1)
    # EB1[t, h, a] = exp(b1[h, a, t]); replicated for the 2-window partition dim
    eb1 = np.exp(b1).transpose(2, 0, 1).reshape(64, 96)
    eb1 = np.tile(eb1, (2, 1)).astype(np.float32)
    # EB2T[a, h, t] = exp(b2[h, t, a])
    eb2t = np.exp(b2).transpose(2, 0, 1).reshape(16, 384).astype(np.float32)
    eb2t = np.tile(eb2t, (3, 1))
    s1 = np.exp(np.minimum(ls1, math.log(100.0))).reshape(NH)
    s2 = np.exp(np.minimum(ls2, math.log(100.0))).reshape(NH)
    scl = np.tile(np.concatenate([s2, s1]).astype(np.float32), (128, 1))
    return eb1, eb2t, np.ascontiguousarray(scl)


def kernel(**inputs):
    kwargs = inputs
    from concourse.bass_utils import run_bass_kernel_spmd

    qkv = np.ascontiguousarray(np.asarray(inputs["qkv"], dtype=np.float32))
    anchor = np.ascontiguousarray(np.asarray(inputs["anchor"], dtype=np.float32))
    table = np.asarray(inputs["table"], dtype=np.float32)
    i_a2w = np.asarray(inputs["index_a2w"]).astype(np.int64)
    i_w2a = np.asarray(inputs["index_w2a"]).astype(np.int64)
    eb1, eb2t, scl = _host_consts(
        table, i_a2w, i_w2a,
        np.asarray(inputs["logit_scale1"], np.float32),
        np.asarray(inputs["logit_scale2"], np.float32),
        np.asarray(inputs["cpb1_w1"], np.float32),
        np.asarray(inputs["cpb1_b1"], np.float32),
        np.asarray(inputs["cpb1_w2"], np.float32),
        np.asarray(inputs["cpb2_w1"], np.float32),
        np.asarray(inputs["cpb2_b1"], np.float32),
        np.asarray(inputs["cpb2_w2"], np.float32),
    )

    in_maps = []
    for c in range(NCORES):
        b = c // 4
        rb = c % 4
        qkv_sh = qkv[b, rb * TOK:(rb + 1) * TOK].reshape(NWR, WS, NWC, WS, C)
        anc_sh = anchor[b, rb * 32:(rb + 1) * 32].reshape(NWR, AWS, NWC, AWS, CO)
        in_maps.append({
            "qkv": np.ascontiguousarray(qkv_sh),
            "anc": np.ascontiguousarray(anc_sh),
            "eb1": eb1, "eb2t": eb2t, "scl": scl,
        })

    nc = _get_nc()
    trace = bool(kwargs.get("_trace"))
    tkw = {}
    if trace:
        tkw = dict(trace=True, tmpdir=kwargs.get("_tmpdir"))
    res = run_bass_kernel_spmd(nc, in_maps, list(range(NCORES)), **tkw)
    results = res.results if hasattr(res, "results") else res
    if trace:
        kernel._last_profile = res

    out = np.empty((B, H * WID, CO), dtype=np.float32)
    for c in range(NCORES):
        b = c // 4
        rb = c % 4
        out[b, rb * TOK:(rb + 1) * TOK] = np.asarray(
            results[c]["out"], dtype=np.float32).reshape(TOK, CO)
    return out
